# revision 29
# baseline (speedup 1.0000x reference)
"""GQA decode-extend kernel for Trainium2 (8 NeuronCores, TP over kv-heads).

Reference quirk (faithful source bug): both attention keys AND values are
repeat_interleave of cache_v (after the fresh v is written at start_pos).
wk / cache_k never influence the output, so we skip them entirely.

v2 design (per core c, kv-head c, q-heads 4c..4c+3):
  - Host ships cache_v ONCE per core, hd-major: cvt[b] = [128 hd, 4096 s]
    (cols 4080:4095 zeroed; fresh v is stitched in on device).
  - q = x @ wq_c (+RoPE), v = x @ wv_c on PE; fresh vT stitched into
    cvt[b][:, 4080:4096].
  - Natural layout for the AV matmul is derived ON DEVICE from cvt[b] with a
    single DMA-XBAR transpose instruction per batch (InstDmaTransposeAnt,
    verified semantic: out[p, m, :] = in^T[m*128+p, :], i.e. chunk m holds
    s = m*128+p on partition p, matching score chunk m = cvt cols
    [m*128, (m+1)*128)).
  - scores chunk-group G (8 chunks): scoresT[s128, 64] = cvt_chunk^T @ qT_b
    -> exp (Act, groups of 8 chunks) -> AV accumulate acc[64, 0:129]
    (col 128 = sumexp via on-device ones column).
  - att = acc[:,0:128] * (1/acc[:,128]); transpose; wproj partial; host sums
    the 8 cores' partials (bf16 partials, f32 host accumulate).
"""

import os
import sys

import numpy as np

sys.path.insert(0, "/opt/trn_rl_repo")

import ml_dtypes

import concourse.bass as bass
import concourse.mybir as mybir
import concourse.tile as tile

# ---------------------------------------------------------------------------
# Workaround for walrus builds that allow only ONE sync wait per instruction
# (2 for EventSemaphore): split excess waits onto standalone EventSemaphore
# instructions inserted before the over-subscribed instruction on the same
# engine queue. Hooks Tile post-sem-assignment + the kernel-tail drain.
_ws_counter = [0]


def _ws_cap(inst):
    return 2 if inst.opcode == "EventSemaphore" else 1


def _ws_split_list(insts):
    out = []
    changed = False
    for inst in insts:
        si = getattr(inst, "sync_info", None)
        waits = list(si.on_wait) if (si and si.on_wait) else []
        cap = _ws_cap(inst)
        if len(waits) > cap:
            changed = True
            keep, extra = waits[:cap], waits[cap:]
            for i in range(0, len(extra), 2):
                _ws_counter[0] += 1
                es = mybir.InstEventSemaphore(
                    name=f"wsplit_{_ws_counter[0]}", ins=[], outs=[]
                )
                es.engine = inst.engine
                es.sync_info = mybir.SyncInfo(
                    on_wait=list(extra[i : i + 2]), on_update=[]
                )
                out.append(es)
            inst.sync_info = mybir.SyncInfo(
                on_wait=keep,
                on_update=list(si.on_update) if si.on_update else [],
            )
        out.append(inst)
    if changed:
        insts[:] = out


_ws_orig_postorder = tile.postorder_instruction_blocks


def _ws_postorder(obb, start_bb_name, postordered_blocks):
    for insts in obb.values():
        _ws_split_list(insts)
    return _ws_orig_postorder(obb, start_bb_name, postordered_blocks)


def _ws_dab(self, tick_clock, wait_clock):
    nc = self.nc
    drain_inst = nc.sync.drain()
    wait_clock.add_sem_waits(
        drain_inst.ins, tile.ScopedClock({None: tick_clock.global_clock})
    )
    raw = drain_inst.ins
    si = raw.sync_info
    if si and si.on_wait and len(si.on_wait) > 1:
        waits = list(si.on_wait)
        raw.sync_info = mybir.SyncInfo(
            on_wait=waits[:1],
            on_update=list(si.on_update) if si.on_update else [],
        )
        extra = waits[1:]
        sp = nc.engines[mybir.EngineType.SP]
        for i in range(0, len(extra), 2):
            _ws_counter[0] += 1
            es = mybir.InstEventSemaphore(
                name=f"wsplit_drain_{_ws_counter[0]}", ins=[], outs=[]
            )
            es.engine = mybir.EngineType.SP
            es.sync_info = mybir.SyncInfo(
                on_wait=list(extra[i : i + 2]), on_update=[]
            )
            sp.add_instruction(es)
    nc.all_engine_barrier()
    assert self.sems is not None
    popped = nc._tile_sem_poison_stack.pop()
    assert popped is self._sem_poison
    nc.clear_and_free_semaphores(list(self.sems.allocated().values()))
    nc.all_engine_barrier()


if not getattr(tile, "_ws_installed", False):
    tile.postorder_instruction_blocks = _ws_postorder
    tile.TileContext._drain_and_barrier = _ws_dab
    tile._ws_installed = True
# ---------------------------------------------------------------------------

from concourse.bass_utils import run_bass_kernel_spmd
from concourse.masks import make_identity

BF16 = ml_dtypes.bfloat16

B, T, SP = 8, 16, 4080
DIM, H, KVH, HD = 4096, 32, 8, 128
S = SP + T            # 4096
NTOK = B * T          # 128
NCORES = 8
NH = H // KVH         # 4 q-heads per core
ECORE = NH * HD       # 512
NCH = S // 128        # 32 s-chunks per batch
DCH = DIM // 128      # 32 d-chunks
SCALE = 1.0 / float(np.sqrt(HD))
CHN = 132             # natural-chunk stride (129 data + 3 pad)
GSZ = 8               # chunks per score/exp group
NG = NCH // GSZ       # 4 groups per batch
LAG = 3  # AV trails scores by LAG groups (software pipeline)

_COMPILED = {}
DEBUG_DUMPS = False
ABLATE = set()


def _build_program(n_rep=1):
    """One SPMD program; per-core data differs via in_maps."""
    nc = bass.Bass()
    f32 = mybir.dt.float32
    bf16 = mybir.dt.bfloat16

    xT_t = nc.dram_tensor("xT", [128, DCH * 128], bf16, kind="ExternalInput")
    wq_t = nc.dram_tensor("wq", [128, DCH * ECORE], bf16, kind="ExternalInput")
    wv_t = nc.dram_tensor("wv", [128, DCH * HD], bf16, kind="ExternalInput")
    wp_t = nc.dram_tensor("wp", [128, NH * DIM], bf16, kind="ExternalInput")
    cos_t = nc.dram_tensor("cosr", [128, 256], f32, kind="ExternalInput")
    sin_t = nc.dram_tensor("sinr", [128, 256], f32, kind="ExternalInput")
    cvt_t = nc.dram_tensor("cvt", [B, 128, S], bf16, kind="ExternalInput")
    out_t = nc.dram_tensor("out", [128, DIM], bf16, kind="ExternalOutput")
    dbg = {}
    if DEBUG_DUMPS:
        dbg["nat"] = nc.dram_tensor(
            "dbg_nat", [B, 128, S], bf16, kind="ExternalOutput"
        )
        dbg["cvt"] = nc.dram_tensor(
            "dbg_cvt", [B, 128, S], bf16, kind="ExternalOutput"
        )
        dbg["att"] = nc.dram_tensor(
            "dbg_att", [B, 64, 128], bf16, kind="ExternalOutput"
        )

    with tile.TileContext(nc) as tc:
        with (
            tc.tile_pool(name="consts", bufs=1) as consts,
            tc.tile_pool(name="wpool", bufs=1) as wpool,
            tc.tile_pool(name="spool", bufs=1) as spool,
            tc.tile_pool(name="cvtp", bufs=3) as cvtp,
            tc.tile_pool(name="natp", bufs=3) as natp,
            tc.tile_pool(name="expp", bufs=4) as expp,
            tc.tile_pool(name="smallp", bufs=4) as smallp,
            tc.tile_pool(name="pp_proj", bufs=1, space="PSUM") as pp_proj,
            tc.tile_pool(name="pp_sc", bufs=3, space="PSUM") as pp_sc,
            tc.tile_pool(name="pp_acc", bufs=2, space="PSUM") as pp_acc,
            tc.tile_pool(name="pp_den", bufs=1, space="PSUM") as pp_den,
            tc.tile_pool(name="pp_tr", bufs=1, space="PSUM") as pp_tr,
        ):
            ident = consts.tile([128, 128], bf16)
            make_identity(nc, ident[:, :])

            for rep in range(n_rep):
                _emit_body(
                    nc, xT_t, wq_t, wv_t, wp_t, cos_t, sin_t, cvt_t, out_t,
                    ident, wpool, spool, cvtp, natp, expp, smallp,
                    pp_proj, pp_sc, pp_acc, pp_den, pp_tr, dbg,
                )

    return nc


def _emit_body(nc, xT_t, wq_t, wv_t, wp_t, cos_t, sin_t, cvt_t, out_t, ident,
               wpool, spool, cvtp, natp, expp, smallp,
               pp_proj, pp_sc, pp_acc, pp_den, pp_tr, dbg={}):
    f32 = mybir.dt.float32
    bf16 = mybir.dt.bfloat16

    # ---- DMA: inputs needed first ----
    xT_sb = wpool.tile([128, DCH * 128], bf16, tag="xT")
    nc.sync.dma_start(out=xT_sb, in_=xT_t[:, :])
    wv_sb = wpool.tile([128, DCH * HD], bf16, tag="wv")
    nc.sync.dma_start(out=wv_sb, in_=wv_t[:, :])
    wq_sb = wpool.tile([128, DCH * ECORE], bf16, tag="wq")
    NSPL = 4
    for s in range(NSPL):
        w = DCH * ECORE // NSPL
        eng = nc.sync if s % 2 == 0 else nc.scalar
        eng.dma_start(
            out=wq_sb[:, s * w : (s + 1) * w], in_=wq_t[:, s * w : (s + 1) * w]
        )
    cos_sb = spool.tile([128, 256], f32, tag="cos")
    nc.sync.dma_start(out=cos_sb, in_=cos_t[:, :])
    sin_sb = spool.tile([128, 256], f32, tag="sin")
    nc.sync.dma_start(out=sin_sb, in_=sin_t[:, :])

    # prefetch first cache batches
    cvt_sbs = {}
    for b in range(2):
        cvt_sbs[b] = cvtp.tile([128, S], bf16, tag="cvt", name=f"cvt{b}")
        nc.scalar.dma_start(out=cvt_sbs[b], in_=cvt_t[b, :, :])

    # DVE TensorTensor has a single HW wait slot; pre-sync the cos/sin DMA
    # sems on DVE with tiny touch copies so the RoPE muls only need the PE
    # wait.
    touch = spool.tile([1, 2], f32, tag="touch")
    nc.vector.tensor_copy(touch[0:1, 0:1], cos_sb[0:1, 0:1])
    nc.vector.tensor_copy(touch[0:1, 1:2], sin_sb[0:1, 0:1])

    ones_sb = spool.tile([128, 1], bf16, tag="ones")
    nc.vector.memset(ones_sb[:, :], 1.0)
    onef_sb = spool.tile([1, 1], f32, tag="onef")
    nc.vector.memset(onef_sb[:, :], 1.0)

    pp_proj_g[0] = pp_proj
    # ---- projections (v first: it gates the per-batch stitch+xbar) ----
    v_ps = pp_proj.tile([128, HD], f32, tag="proj")
    for i in range(DCH):
        nc.tensor.matmul(
            v_ps[:, :],
            xT_sb[:, i * 128 : (i + 1) * 128],
            wv_sb[:, i * HD : (i + 1) * HD],
            start=(i == 0),
            stop=(i == DCH - 1),
        )
    vfN_sb = spool.tile([128, HD], bf16, tag="vfN")
    nc.scalar.copy(vfN_sb[:, :], v_ps[:, :])
    ptv = pp_tr.tile([128, 128], bf16, tag="tr")
    nc.tensor.transpose(ptv[:, :], vfN_sb[:, :], ident[:, :])
    vfT_sb = spool.tile([128, 128], bf16, tag="vfT")
    nc.scalar.copy(vfT_sb[:, :], ptv[:, :])

    q_ps = pp_proj.tile([128, ECORE], f32, tag="proj")
    for i in range(DCH):
        nc.tensor.matmul(
            q_ps[:, :],
            xT_sb[:, i * 128 : (i + 1) * 128],
            wq_sb[:, i * ECORE : (i + 1) * ECORE],
            start=(i == 0),
            stop=(i == DCH - 1),
        )

    # ---- RoPE on q (pairs are adjacent elements) ----
    q_rope = spool.tile([128, ECORE], bf16, tag="qrope")
    qp = q_ps.rearrange("p (i two) -> p i two", two=2)
    rp = q_rope.rearrange("p (i two) -> p i two", two=2)
    ua = smallp.tile([128, 256], f32, tag="u")
    ub = smallp.tile([128, 256], f32, tag="u")
    nc.vector.tensor_mul(ua[:, :], qp[:, :, 0], cos_sb[:, :])
    nc.vector.tensor_mul(ub[:, :], qp[:, :, 1], sin_sb[:, :])
    nc.vector.tensor_sub(rp[:, :, 0], ua[:, :], ub[:, :])
    uc = smallp.tile([128, 256], f32, tag="u")
    ud = smallp.tile([128, 256], f32, tag="u")
    nc.vector.tensor_mul(uc[:, :], qp[:, :, 0], sin_sb[:, :])
    nc.vector.tensor_mul(ud[:, :], qp[:, :, 1], cos_sb[:, :])
    nc.vector.tensor_add(rp[:, :, 1], uc[:, :], ud[:, :])

    # ---- qT (per head): col layout b*64 + h*16 + t ----
    qT_sb = spool.tile([128, ECORE], bf16, tag="qT")
    qT4 = qT_sb.rearrange("p (bb hh t) -> p bb hh t", bb=B, hh=NH)
    for h in range(NH):
        pt = pp_tr.tile([128, 128], bf16, tag="tr")
        nc.tensor.transpose(
            pt[:, :], q_rope[:, h * 128 : (h + 1) * 128], ident[:, :]
        )
        # pt cols = global token (b*16+t) -> scatter to (b, h, t)
        nc.scalar.copy(
            qT4[:, :, h, :], pt.rearrange("p (bb t) -> p bb t", bb=B)
        )

    # outT col layout: h*128 + b*16 + t (wproj lhsT per head is contiguous)
    outT_sb = spool.tile([128, B * NH * T], bf16, tag="outT")
    oT4 = outT_sb.rearrange("p (hh bb t) -> p hh bb t", hh=NH, bb=B)

    nats = {}

    def issue_stitch_xbar(b):
        # stitch fresh vT into true s-slots 4080:4096 (Pool: keeps the Act
        # queue free for exp and the SP queue unblocked)
        nc.gpsimd.tensor_copy(
            cvt_sbs[b][:, SP:S], vfT_sb[:, b * T : (b + 1) * T]
        )
        # derive natural layout: one XBAR transpose per batch
        # natc[p, c*128 + l] = cvt^T[c*128 + p, l] = v[s=c*128+p, hd=l]
        natc = natp.tile([128, S], bf16, tag="nat", name=f"nat{b}")
        nc3 = natc.rearrange("p (c l) -> p c l", l=128)
        if "xbar" not in ABLATE:
            nc.sync.dma_start(out=nc3, in_=cvt_sbs[b][:, :], transpose=True)
        else:
            nats[b] = cvt_sbs[b]  # timing-only substitute; results wrong
        nats[b] = natc
        if dbg:
            nc.sync.dma_start(out=dbg["nat"][b, :, :], in_=natc)
            nc.sync.dma_start(out=dbg["cvt"][b, :, :], in_=cvt_sbs[b])

    for b in range(2):
        issue_stitch_xbar(b)

    # ---- attention: software-pipelined over (batch, group) ----
    pend = []  # (b, g, ex_tile, acc_tile, nat_tile)

    def issue_av(b, g, ex, accden, natc):
        if "av" in ABLATE:
            return
        acc, den = accden
        for j in range(GSZ):
            c = g * GSZ + j
            nc.tensor.matmul(
                acc[:, :],
                ex[:, j * 64 : (j + 1) * 64],
                natc[:, c * 128 : (c + 1) * 128],
                start=(c == 0),
                stop=(c == NCH - 1),
            )
        # sumexp partials: den[0, j*64+ht] accumulates chunk-residue j over
        # groups; folded over j at finish. Separate PSUM bank: a start=True
        # here must not reset the data accumulator.
        nc.tensor.matmul(
            den[:, :],
            ones_sb[:, :],
            ex[:, :],
            start=(g == 0),
            stop=(g == NG - 1),
        )

    accs = {}
    fin2 = []
    for b in range(B):
        cvt_sb = cvt_sbs[b]
        if b + 2 < B:
            cvt_sbs[b + 2] = cvtp.tile(
                [128, S], bf16, tag="cvt", name=f"cvt{b + 2}"
            )
            nc.sync.dma_start(out=cvt_sbs[b + 2], in_=cvt_t[b + 2, :, :])
            issue_stitch_xbar(b + 2)
        if b == 2:
            # wp arrives well before the epilogue; issue mid-stream
            wp_sb = wpool.tile([128, NH * DIM], bf16, tag="wp")
            hw = NH * DIM // 2
            nc.sync.dma_start(out=wp_sb[:, 0:hw], in_=wp_t[:, 0:hw])
            nc.scalar.dma_start(out=wp_sb[:, hw:], in_=wp_t[:, hw:])

        if "av" not in ABLATE:
            acc = pp_acc.tile([64, 128], f32, tag="acc")
            den = pp_den.tile([1, GSZ * 64], f32, tag="den")
            accs[b] = (acc, den)
        else:
            accs[b] = None
        qrhs = qT_sb[:, b * 64 : (b + 1) * 64]

        for g in range(NG):
            scp = pp_sc.tile([128, GSZ * 64], f32, tag="sc")
            if "scores" not in ABLATE:
                for j in range(GSZ):
                    c = g * GSZ + j
                    nc.tensor.matmul(
                        scp[:, j * 64 : (j + 1) * 64],
                        cvt_sb[:, c * 128 : (c + 1) * 128],
                        qrhs,
                        start=True,
                        stop=True,
                    )
            ex = expp.tile([128, GSZ * 64], bf16, tag="ex", name=f"ex{b}_{g}")
            if "exp" not in ABLATE:
                nc.scalar.activation(
                    ex[:, :], scp[:, :], mybir.ActivationFunctionType.Exp,
                    scale=SCALE,
                )
            pend.append((b, g, ex))
            if len(pend) > LAG:
                pb, pg, pex = pend.pop(0)
                issue_av(pb, pg, pex, accs[pb], nats[pb])
                if pg == NG - 1:
                    if "av" not in ABLATE:
                        attb = _finish_part1(nc, pb, accs.pop(pb), onef_sb,
                                             smallp, pp_tr, dbg)
                        fin2.append((pb, attb, 2))
                    nats.pop(pb)
                    cvt_sbs.pop(pb)
                fin2 = [(fb, fa, k - 1) for fb, fa, k in fin2]
                while fin2 and fin2[0][2] <= 0:
                    fb, fa, _ = fin2.pop(0)
                    _finish_part2(nc, fb, fa, oT4, ident, pp_tr)

    while pend:
        pb, pg, pex = pend.pop(0)
        issue_av(pb, pg, pex, accs[pb], nats[pb])
        if pg == NG - 1 and "av" not in ABLATE:
            attb = _finish_part1(nc, pb, accs.pop(pb), onef_sb, smallp,
                                 pp_tr, dbg)
            fin2.append((pb, attb, 0))
    for fb, fa, _ in fin2:
        _finish_part2(nc, fb, fa, oT4, ident, pp_tr)
    if "av" in ABLATE:
        nc.vector.memset(outT_sb[:, :], 0.0)

    # ---- output projection (partial; host sums cores) ----
    out_sb = spool.tile([128, DIM], bf16, tag="outsb")
    for nt in range(DIM // 512):
        po = pp_sc.tile([128, 512], f32, tag="sc")
        for h in range(NH):
            nc.tensor.matmul(
                po[:, :],
                outT_sb[:, h * 128 : (h + 1) * 128],
                wp_sb[:, h * DIM + nt * 512 : h * DIM + (nt + 1) * 512],
                start=(h == 0),
                stop=(h == NH - 1),
            )
        nc.vector.tensor_copy(out_sb[:, nt * 512 : (nt + 1) * 512], po[:, :])
        nc.sync.dma_start(
            out=out_t[:, nt * 512 : (nt + 1) * 512],
            in_=out_sb[:, nt * 512 : (nt + 1) * 512],
        )


pp_proj_g = [None]


def _finish_part1(nc, b, accden, onef_sb, smallp, pp_tr, dbg={}):
    bf16 = mybir.dt.bfloat16
    f32 = mybir.dt.float32
    acc, den = accden
    den_sb = smallp.tile([1, 64], f32, tag="densb")
    nc.vector.tensor_reduce(
        den_sb[:, :],
        den.rearrange("p (j h) -> p h j", j=GSZ),
        axis=mybir.AxisListType.X,
        op=mybir.AluOpType.add,
    )
    denT = pp_proj_g[0].tile([64, 1], f32, tag="proj", name=f"denT{b}")
    nc.tensor.transpose(denT[:, :], den_sb[0:1, 0:64], onef_sb[:, :])
    rcp = smallp.tile([64, 1], f32, tag="rcp")
    nc.vector.reciprocal(rcp[:, :], denT[:, :])
    attb = smallp.tile([64, 128], bf16, tag="attb")
    nc.vector.tensor_scalar_mul(attb[:, :], acc[:, :], rcp[:, :])
    if dbg:
        nc.sync.dma_start(out=dbg["att"][b, :, :], in_=attb)
    return attb


def _finish_part2(nc, b, attb, oT4, ident, pp_tr):
    bf16 = mybir.dt.bfloat16
    pt2 = pp_tr.tile([128, 64], bf16, tag="tr")
    nc.tensor.transpose(pt2[:, :], attb[:, :], ident[0:64, 0:64])
    # pt2 cols = (h,t) for batch b -> scatter to (h, b, t)
    nc.vector.tensor_copy(
        oT4[:, :, b, :], pt2.rearrange("p (hh t) -> p hh t", hh=NH)
    )


def _prep_inputs(x, wq, wv, cache_v):
    """Host-side shard + layout prep. Returns list of 8 in_maps."""
    x2d = np.ascontiguousarray(x.reshape(NTOK, DIM)).astype(np.float32)
    # xT tiled: [p, i*128+t] = x2d[t, i*128+p]
    xT = np.ascontiguousarray(
        x2d.reshape(NTOK, DCH, 128).transpose(2, 1, 0).reshape(128, DCH * 128)
    ).astype(BF16)

    in_maps = []
    for c in range(NCORES):
        wq_c = wq[:, c * ECORE : (c + 1) * ECORE].astype(np.float32)
        wq_l = np.ascontiguousarray(
            wq_c.reshape(DCH, 128, ECORE).transpose(1, 0, 2).reshape(128, DCH * ECORE)
        ).astype(BF16)
        wv_c = wv[:, c * HD : (c + 1) * HD].astype(np.float32)
        wv_l = np.ascontiguousarray(
            wv_c.reshape(DCH, 128, HD).transpose(1, 0, 2).reshape(128, DCH * HD)
        ).astype(BF16)
        wp_c = np.ascontiguousarray(wproj_g[c * ECORE : (c + 1) * ECORE, :]).astype(
            np.float32
        )
        wp_l = np.ascontiguousarray(
            wp_c.reshape(NH, 128, DIM).transpose(1, 0, 2).reshape(128, NH * DIM)
        ).astype(BF16)

        # cvt[b] = [hd, s]: cache rows 0..4079 at their true s; fresh slots 0
        cv = cache_v[:, :SP, c, :]  # [B, 4080, 128]
        cvt = np.zeros((B, 128, S), dtype=BF16)
        cvt[:, :, :SP] = cv.transpose(0, 2, 1).astype(BF16)

        in_maps.append(
            {
                "xT": xT,
                "wq": wq_l,
                "wv": wv_l,
                "wp": wp_l,
                "cosr": cos_rep_g,
                "sinr": sin_rep_g,
                "cvt": np.ascontiguousarray(cvt),
            }
        )
    return in_maps


# globals filled by kernel() before _prep_inputs uses them
wproj_g = None
cos_rep_g = None
sin_rep_g = None


def kernel(
    x,
    wq,
    wk,
    wv,
    wproj,
    cache_k,
    cache_v,
    freqs_cos,
    freqs_sin,
    mask,
    start_pos,
    _trace=False,
):
    global wproj_g, cos_rep_g, sin_rep_g
    assert int(start_pos) == SP
    x = np.asarray(x, dtype=np.float32)
    wproj_g = np.asarray(wproj, dtype=np.float32)
    fc = np.asarray(freqs_cos, dtype=np.float32)
    fs = np.asarray(freqs_sin, dtype=np.float32)
    # replicate freqs: [p, j] = f[p % 16, j % 64]
    cos_rep_g = np.ascontiguousarray(np.tile(fc, (B, NH))).astype(np.float32)
    sin_rep_g = np.ascontiguousarray(np.tile(fs, (B, NH))).astype(np.float32)

    in_maps = _prep_inputs(
        x, np.asarray(wq, np.float32), np.asarray(wv, np.float32),
        np.asarray(cache_v, np.float32),
    )

    if "prog" not in _COMPILED:
        _COMPILED["prog"] = _build_program()
    nc = _COMPILED["prog"]

    res = run_bass_kernel_spmd(
        nc, in_maps, core_ids=list(range(NCORES)), trace=_trace
    )
    out = np.zeros((NTOK, DIM), dtype=np.float32)
    for r in res.results:
        out += r["out"].astype(np.float32)
    if _trace:
        kernel.last_results = res
    return out.reshape(B, T, DIM)
